# revision 1
# baseline (speedup 1.0000x reference)
"""Trainium2 Bass kernel for nn_AttentionCombine.

Self-contained: builds an SPMD Bass graph (same graph on 8 NeuronCores),
shards inputs data-parallel over the batch dim (4 images / 256 objects per
core), runs via run_bass_kernel_spmd, and reassembles the full output.

Per-core dataflow:
  - cnn feature maps for 2 images packed as [128 part, 25600] f32 in SBUF
    (64 channels per image, image A on partitions 0-63, image B on 64-127)
  - bilinear sampling of 32 contour points x 4 corners via gpsimd ap_gather
    (host precomputes clamped flat offsets + bilinear weights, zeroed when
    out of bounds)
  - corner-weighted sum on VectorE -> contour features, written into
    K-tile layout [(channel, point-pair) x objects] (cross-partition half
    moved with a SBUF->SBUF DMA)
  - conv1d == GEMM over K=(66 ch x 32 pts) on TensorE (bf16, fp32 psum)
  - + positional embedding (host-gathered, device add)
  - qk GEMM (attention in_proj, p_w/sqrt(hd) folded into q rows on host)
  - attention: per image, 4 accumulating K=128 matmuls (each contracts a
    128-channel chunk = two heads at once; valid because the per-head
    combine weights are already folded into q)
  - sigmoid on ScalarE, DMA out
"""
import os
import sys

for _p in ("/opt/trn_rl_repo", "/root/.axon_site/_ro/trn_rl_repo"):
    if os.path.isdir(_p) and _p not in sys.path:
        sys.path.append(_p)

import numpy as np
from contextlib import ExitStack

from concourse import bacc, mybir
from concourse.tile import TileContext
from concourse.bass_utils import run_bass_kernel_spmd

F32 = mybir.dt.float32
BF16 = mybir.dt.bfloat16
I16 = mybir.dt.int16

# Problem constants (hardcoded per spec)
B, C, H, W = 32, 64, 160, 160
IMG_HW = 640
N_OBJ = 2048
NUM_POINTS = 128
STRIDE = 4
P = NUM_POINTS // STRIDE  # 32 sampled points
NE = 512                  # n_embd
HEADS = 8
PATCH = 16
T = 64                    # objects per image
N_CORES = 8
IMGS_PER_CORE = B // N_CORES      # 4
OBJS_PER_CORE = N_OBJ // N_CORES  # 256
HW_PIX = H * W                    # 25600

_MODEL_CACHE = {}


def build_model():
    if "nc" in _MODEL_CACHE:
        return _MODEL_CACHE["nc"]
    nc = bacc.Bacc("TRN2", target_bir_lowering=False, debug=False)
    AL = mybir.AluOpType
    AF = mybir.ActivationFunctionType

    fm_e = nc.declare_dram_parameter("fm", [2, 128, HW_PIX], BF16, isOutput=False)
    idx_e = nc.declare_dram_parameter("idx", [2, 128, 256], I16, isOutput=False)
    wrep_e = nc.declare_dram_parameter("wrep", [2, 2, 8192], BF16, isOutput=False)
    ktn_e = nc.declare_dram_parameter("ktn", [128, 256], BF16, isOutput=False)
    cw_e = nc.declare_dram_parameter("cw", [128, 17 * 4 * 128], BF16, isOutput=False)
    aw_e = nc.declare_dram_parameter("aw", [128, 4 * 8 * 128], BF16, isOutput=False)
    posb_e = nc.declare_dram_parameter("posb", [128, 4 * 256], F32, isOutput=False)
    ab_e = nc.declare_dram_parameter("ab", [128, 8], F32, isOutput=False)
    out_e = nc.declare_dram_parameter("out", [4, 64, 64], F32, isOutput=True)

    with TileContext(nc) as tc, ExitStack() as ctx:
        const = ctx.enter_context(tc.tile_pool(name="const", bufs=1))
        cw_sb = const.tile([128, 17 * 4 * 128], BF16, tag="cw")
        aw_sb = const.tile([128, 4 * 8 * 128], BF16, tag="aw")
        posb_sb = const.tile([128, 1024], F32, tag="posb")
        ab_sb = const.tile([128, 8], F32, tag="ab")
        nc.sync.dma_start(cw_sb[:], cw_e[:])
        nc.sync.dma_start(aw_sb[:], aw_e[:])
        nc.sync.dma_start(posb_sb[:], posb_e[:])
        nc.sync.dma_start(ab_sb[:], ab_e[:])

        ktp = ctx.enter_context(tc.tile_pool(name="kt", bufs=1))
        KT = ktp.tile([128, 17 * 256], BF16, tag="kt")
        nc.sync.dma_start(KT[:, 16 * 256:17 * 256], ktn_e[:])

        # MAP holds, as bf16: [0:25600] = the two images' maps (x-pair units
        # at even alignment) and [25600:51200] = the same maps shifted one
        # pixel left (x-pair units at odd alignment). Gathered as uint32
        # units of two x-adjacent pixels -> halves the gather index count.
        fmp = ctx.enter_context(tc.tile_pool(name="fmp", bufs=1))
        MAP = fmp.tile([128, 2 * HW_PIX], BF16, tag="fm")
        MAPu = MAP[:].bitcast(mybir.dt.uint32)

        idxp = ctx.enter_context(tc.tile_pool(name="idxp", bufs=2))
        gp = ctx.enter_context(tc.tile_pool(name="gp", bufs=1))
        wp = ctx.enter_context(tc.tile_pool(name="wp", bufs=1))
        fp = ctx.enter_context(tc.tile_pool(name="fp", bufs=1))
        sp = ctx.enter_context(tc.tile_pool(name="sp", bufs=2))
        cfp = ctx.enter_context(tc.tile_pool(name="cfp", bufs=2))
        qkp = ctx.enter_context(tc.tile_pool(name="qkp", bufs=2))
        attp = ctx.enter_context(tc.tile_pool(name="attp", bufs=4))
        psp = ctx.enter_context(tc.tile_pool(name="psp", bufs=6, space="PSUM"))
        psap = ctx.enter_context(tc.tile_pool(name="psap", bufs=2, space="PSUM"))

        cwv = cw_sb[:].rearrange("p (j o m) -> p j o m", j=17, o=4, m=128)
        awv = aw_sb[:].rearrange("p (k m c) -> p k m c", k=4, m=8, c=128)
        posv = posb_sb[:].rearrange("p (o n) -> p o n", o=4, n=256)
        KTj = KT[:].rearrange("p (j n) -> p j n", j=17, n=256)
        KTw = KT[:, 0:16 * 256].rearrange("p (j a b t) -> p j a b t", j=16, a=2, b=2, t=64)

        for pp in range(2):
            with nc.named_scope(f"fm_dma_{pp}"):
                nc.sync.dma_start(MAP[:, 0:HW_PIX], fm_e[pp])
                # shifted copy: element i of the shifted half = pixel i+1
                nc.sync.dma_start(MAP[:, HW_PIX:2 * HW_PIX - 1],
                                  MAP[:, 1:HW_PIX])
                nc.gpsimd.memset(MAP[:, 2 * HW_PIX - 1:2 * HW_PIX], 0.0)
            IDX = idxp.tile([128, 256], I16, tag="idx")
            nc.sync.dma_start(IDX[:], idx_e[pp])

            # gather x-pair units for both bilinear rows of every point
            G = gp.tile([128, 4096], mybir.dt.uint32, tag="g")
            with nc.named_scope(f"gather_{pp}"):
                nc.gpsimd.ap_gather(
                    G[:], MAPu, IDX[:],
                    channels=128, num_elems=HW_PIX, d=1, num_idxs=4096)
            Gb = G[:].bitcast(BF16)           # [128, 8192] (.., y, j, s, t, lr)

            WR = wp.tile([128, 8192], BF16, tag="w")
            nc.sync.dma_start(WR[0:64, :], wrep_e[pp, 0].partition_broadcast(64))
            nc.sync.dma_start(WR[64:128, :], wrep_e[pp, 1].partition_broadcast(64))
            nc.vector.tensor_tensor(Gb, Gb, WR[:], AL.mult)

            # corner reduction: first the left/right slots, then the two rows
            T1 = fp.tile([128, 4096], BF16, tag="t1")
            Gv = G[:].bitcast(BF16).rearrange("p (f l) -> p f l", f=4096, l=2)
            nc.vector.tensor_tensor(T1[:], Gv[:, :, 0], Gv[:, :, 1], AL.add)
            FEATS = fp.tile([128, 2048], BF16, tag="feats")
            nc.vector.tensor_tensor(FEATS[:], T1[:, 0:2048], T1[:, 2048:4096],
                                    AL.add)

            F4 = FEATS[:].rearrange("p (j s t) -> p j s t", j=16, s=2, t=64)
            for hh in range(2):
                lo = 64 * hh       # partitions of this image's channels
                dlo = 64 * (1 - hh)
                # same-partition block (point parity s == hh)
                nc.vector.tensor_copy(KTw[lo:lo + 64, :, pp, hh, :],
                                      F4[lo:lo + 64, :, hh, :])
                # cross-partition block via staging + SBUF->SBUF DMA
                STG = sp.tile([128, 1024], BF16, tag="stg")
                nc.vector.tensor_copy(STG[lo:lo + 64, :], F4[lo:lo + 64, :, 1 - hh, :])
                nc.sync.dma_start(KTw[dlo:dlo + 64, :, pp, hh, :], STG[lo:lo + 64, :])

            # GEMM1 (conv) on this pair's 128 object-columns
            nsl = slice(pp * 128, pp * 128 + 128)
            CF = cfp.tile([128, 512], BF16, tag="cf")
            for o in range(4):
                ps = psp.tile([128, 128], F32, tag="ps")
                for j in range(17):
                    nc.tensor.matmul(ps[:], lhsT=cwv[:, j, o, :], rhs=KTj[:, j, nsl],
                                     start=(j == 0), stop=(j == 16))
                nc.vector.tensor_tensor(CF[:, o * 128:(o + 1) * 128], ps[:],
                                        posv[:, o, nsl], AL.add)

            # GEMM2 (attention in_proj)
            QK = qkp.tile([128, 1024], BF16, tag="qk")
            for m8 in range(8):
                ps = psp.tile([128, 128], F32, tag="ps")
                for k in range(4):
                    nc.tensor.matmul(ps[:], lhsT=awv[:, k, m8, :],
                                     rhs=CF[:, k * 128:(k + 1) * 128],
                                     start=(k == 0), stop=(k == 3))
                nc.scalar.activation(QK[:, m8 * 128:(m8 + 1) * 128], ps[:],
                                     AF.Identity, bias=ab_sb[:, m8:m8 + 1])

            # attention per image: att[t, s] = sum_h (p_w[h]/8) Q_h K_h^T
            # head weighting is folded into q rows, so a K=128 contraction
            # sums the two heads living on the two 64-partition halves of
            # each 128-channel chunk: 4 accumulating matmuls per image.
            for hh in range(2):
                ps = psap.tile([64, 64], F32, tag="psa")
                for qc in range(4):
                    lhsT = QK[:, qc * 128 + hh * 64: qc * 128 + hh * 64 + 64]
                    rhs = QK[:, (4 + qc) * 128 + hh * 64: (4 + qc) * 128 + hh * 64 + 64]
                    nc.tensor.matmul(ps[:], lhsT=lhsT, rhs=rhs,
                                     start=(qc == 0), stop=(qc == 3))
                ATT = attp.tile([64, 64], F32, tag="att")
                nc.scalar.activation(ATT[:], ps[:], AF.Sigmoid)
                nc.sync.dma_start(out_e[2 * pp + hh], ATT[:])

    nc.compile()
    _MODEL_CACHE["nc"] = nc
    return nc


def host_prep(inputs):
    """Host-side sharding + layout prep. Returns list of 8 per-core input maps."""
    import ml_dtypes
    bf = ml_dtypes.bfloat16

    cnn = np.ascontiguousarray(np.asarray(inputs["cnn_feature"], dtype=np.float32))
    contours = np.asarray(inputs["contours"], dtype=np.float32)
    ct_01 = np.asarray(inputs["ct_01"])
    ct_img_idx = np.asarray(inputs["ct_img_idx"])
    ct_ind = np.asarray(inputs["ct_ind"])
    h = int(inputs["h"]); w = int(inputs["w"])
    conv_w = np.asarray(inputs["conv_w"], dtype=np.float32)
    conv_b = np.asarray(inputs["conv_b"], dtype=np.float32)
    attn_w = np.asarray(inputs["attn_w"], dtype=np.float32)
    attn_b = np.asarray(inputs["attn_b"], dtype=np.float32)
    p_w = np.asarray(inputs["p_w"], dtype=np.float32)
    pos_embed = np.asarray(inputs["pos_embed"], dtype=np.float32)

    assert bool(np.all(ct_01)), "kernel requires ct_01 all ones"
    assert bool(np.all(ct_img_idx == np.repeat(np.arange(B, dtype=ct_img_idx.dtype), T)))

    cs = np.ascontiguousarray(contours[:, ::STRIDE])          # [N, 32, 2]
    px = cs[..., 0] * (float(W) / w) - 0.5
    py = cs[..., 1] * (float(H) / h) - 0.5
    x0 = np.floor(px); y0 = np.floor(py)
    wx1 = px - x0; wx0 = 1.0 - wx1
    wy1 = py - y0; wy0 = 1.0 - wy1
    # x-pair gather units: unit = flat//2 + (flat%2)*HW_PIX//2 covers pixels
    # (x0c, x0c+1) of one row (odd-aligned units come from the shifted map
    # copy). Two units per point (rows y0, y1); per-unit (left, right) slot
    # weights, zeroed for out-of-bounds corners; x0 == -1 remaps to x0c = 0
    # with the x0+1 corner weight landing on the left slot.
    offs = np.zeros((2, N_OBJ, P), np.int16)   # [row, n, p] unit index
    wgtL = np.zeros((2, N_OBJ, P), np.float32)
    wgtR = np.zeros((2, N_OBJ, P), np.float32)
    vx0 = x0 >= 0
    vx1 = (x0 + 1) < W
    x0c = np.maximum(x0, 0).astype(np.int32)
    for ri, (yy, wy) in enumerate(((y0, wy0), (y0 + 1, wy1))):
        vy = (yy >= 0) & (yy < H)
        yc = np.clip(yy, 0, H - 1).astype(np.int32)
        flat = yc * W + x0c
        offs[ri] = (flat // 2 + (flat % 2) * (HW_PIX // 2)).astype(np.int16)
        wl = wx0 * wy * vy * vx0
        wr = wx1 * wy * vy * vx1
        wgtL[ri] = np.where(vx0, wl, wr).astype(np.float32)
        wgtR[ri] = np.where(vx0, wr, 0.0).astype(np.float32)

    normed = cs / np.array([w, h], np.float32)                # [N, 32, 2]

    ct_x = (ct_ind % W).astype(np.int64) * PATCH // W
    ct_y = (ct_ind // W).astype(np.int64) * PATCH // H
    posb_full = pos_embed[:, ct_y, ct_x] + conv_b[:, None]    # [512, N]

    s = np.ones(2 * NE, np.float32)
    s[:NE] = np.repeat(p_w[0, :, 0], NE // HEADS) / np.sqrt(np.float32(NE // HEADS))
    aw_t = (attn_w * s[:, None]).T                            # [512, 1024] (k, m)
    ab = attn_b * s                                           # [1024]

    # conv_w K-tiles -> cwT [128, 17*4*128]
    cw = np.zeros((17, 128, 512), np.float32)
    q = np.arange(128)
    for j in range(16):
        cw[j] = conv_w[:, q % 64, 2 * j + q // 64].T          # [128, 512]
    q64 = np.arange(64)
    cw[16, :64] = conv_w[:, 64 + q64 // 32, q64 % 32].T
    cwT = cw.reshape(17, 128, 4, 128).transpose(1, 0, 2, 3).reshape(128, 17 * 4 * 128)

    awT = aw_t.reshape(4, 128, 8, 128).transpose(1, 0, 2, 3).reshape(128, 4 * 8 * 128)
    abT = np.ascontiguousarray(ab.reshape(8, 128).T)          # [128, 8]

    # stream index decomposition (f = row*2048 + j*128 + s*64 + t)
    f = np.arange(4096)
    f_row = f // 2048
    r = f % 2048
    f_j = r // 128
    f_s = (r % 128) // 64
    f_t = r % 64
    f_p = 2 * f_j + f_s

    in_maps = []
    for core in range(N_CORES):
        imgs = [IMGS_PER_CORE * core + i for i in range(IMGS_PER_CORE)]
        nbase = OBJS_PER_CORE * core
        fm = cnn[imgs].reshape(IMGS_PER_CORE, C, HW_PIX).reshape(2, 128, HW_PIX)

        idx = np.zeros((2, 128, 256), np.int16)
        wrep = np.zeros((2, 2, 8192), np.float32)
        for pp in range(2):
            for hh in range(2):
                m = 2 * pp + hh
                n = nbase + T * m + f_t
                si = offs[f_row, n, f_p]
                wrapped = si.reshape(256, 16).T               # [16, 256]
                idx[pp, 64 * hh:64 * hh + 64] = np.tile(wrapped, (4, 1))
                wrep[pp, hh, 0::2] = wgtL[f_row, n, f_p]
                wrep[pp, hh, 1::2] = wgtR[f_row, n, f_p]

        # ktnorm [128, 256]: q<64: (coord=q//32, p=q%32); cols m*64+t
        ktn = np.zeros((128, 256), np.float32)
        ncols = nbase + np.arange(256)
        ktn[:64] = normed[ncols][:, np.arange(64) % 32, np.arange(64) // 32].T

        posbT = np.ascontiguousarray(
            posb_full[:, nbase:nbase + 256].reshape(4, 128, 256)
            .transpose(1, 0, 2).reshape(128, 1024))

        in_maps.append({
            "fm": np.ascontiguousarray(fm).astype(bf),
            "idx": idx,
            "wrep": wrep.astype(bf),
            "ktn": ktn.astype(bf),
            "cw": cwT.astype(bf),
            "aw": awT.astype(bf),
            "posb": posbT.astype(np.float32),
            "ab": abT.astype(np.float32),
        })
    return in_maps


def run(in_maps, trace=False, **kw):
    nc = build_model()
    res = run_bass_kernel_spmd(nc, in_maps, core_ids=list(range(N_CORES)),
                               trace=trace, **kw)
    return res


def kernel(**inputs):
    in_maps = host_prep(inputs)
    res = run(in_maps)
    out = np.concatenate([res.results[i]["out"] for i in range(N_CORES)], axis=0)
    return out.astype(np.float32)



# revision 12
# speedup vs baseline: 2.3656x; 2.3656x over previous
"""Trainium2 Bass kernel for nn_AttentionCombine.

Self-contained: builds an SPMD Bass graph (same graph on 8 NeuronCores),
shards inputs data-parallel over the batch dim (4 images / 256 objects per
core), runs via run_bass_kernel_spmd, and reassembles the full output.

Gather strategy (v2): instead of loading full feature maps into SBUF and
using gpsimd ap_gather (~140us per gather), the host stores each image's
feature map in HBM as 2x2-pixel-tile blocks of 512B ([yoff(2), xoff(2),
ch(64)] bf16), replicated at the 4 (y,x) alignment parities. Any bilinear
2x2 footprint is then exactly ONE block, so one dma_gather(transpose=True)
index per contour point pulls all 4 corners x 64 channels straight from
HBM into SBUF in [partition=(xoff,ch), free=(yoff, point)] layout - the
exact layout the conv GEMM K-tiles need.

Per-core dataflow:
  - 4x dma_gather (one per image, 2048 idxs x 512B) on the SWDGE/pool path
  - corner-weight multiply on VectorE (weights host-computed, broadcast
    to the 64-partition halves), y-corner add in the free dim
  - x-corner add via one small SBUF->SBUF stage DMA + VectorE add,
    written into K-tile layout [(pt-parity, channel) x (j, img, obj)]
  - conv1d == GEMM over K=(66ch x 32pts) on TensorE (bf16, fp32 psum)
  - + positional embedding (host-gathered, device add)
  - qk GEMM (attention in_proj, p_w/sqrt(hd) folded into q rows on host)
  - attention per image: 4 accumulating K=128 matmuls
  - sigmoid on ScalarE, DMA out
"""
import os
import sys

for _p in ("/opt/trn_rl_repo", "/root/.axon_site/_ro/trn_rl_repo"):
    if os.path.isdir(_p) and _p not in sys.path:
        sys.path.append(_p)

import numpy as np
from contextlib import ExitStack

from concourse import bacc, mybir
from concourse.tile import TileContext
from concourse.bass_utils import run_bass_kernel_spmd

F32 = mybir.dt.float32
BF16 = mybir.dt.bfloat16
I16 = mybir.dt.int16

# Problem constants (hardcoded per spec)
B, C, H, W = 32, 64, 160, 160
IMG_HW = 640
N_OBJ = 2048
NUM_POINTS = 128
STRIDE = 4
P = NUM_POINTS // STRIDE  # 32 sampled points
NE = 512                  # n_embd
HEADS = 8
PATCH = 16
T = 64                    # objects per image
N_CORES = 8
IMGS_PER_CORE = B // N_CORES      # 4
OBJS_PER_CORE = N_OBJ // N_CORES  # 256
NPTS = P * T                      # 2048 gather points per image
NBLK = 4 * (H // 2) * (W // 2)    # 25600 tile-blocks per image
CHUNK = 512                       # idxs per dma_gather call
NCHUNK = NPTS // CHUNK            # 4 calls per image

_MODEL_CACHE = {}


def build_model():
    if "nc" in _MODEL_CACHE:
        return _MODEL_CACHE["nc"]
    nc = bacc.Bacc("TRN2", target_bir_lowering=False, debug=False)
    AL = mybir.AluOpType
    AF = mybir.ActivationFunctionType

    # fmb: per image, NBLK blocks of 256 bf16 = [yoff, xoff, ch]
    fmb_e = nc.declare_dram_parameter("fmb", [IMGS_PER_CORE, NBLK, 256], BF16, isOutput=False)
    # gather split into NCHUNK calls/image (SWDGE desc ring caps one call at
    # ~1008 idxs for 512B elems: rx descs/engine = n*2/16+2 must be <= 128)
    idx_e = nc.declare_dram_parameter(
        "idx", [IMGS_PER_CORE, 128, NCHUNK * (CHUNK // 16)], I16, isOutput=False)
    # wrep[xoff_half, img, (chunk, yoff, i512)]  (global i = sp*1024 + j*64 + t)
    wrep_e = nc.declare_dram_parameter("wrep", [2, IMGS_PER_CORE, 2 * NPTS], BF16, isOutput=False)
    ktn_e = nc.declare_dram_parameter("ktn", [128, 256], BF16, isOutput=False)
    cw_e = nc.declare_dram_parameter("cw", [128, 17 * 4 * 128], BF16, isOutput=False)
    aw_e = nc.declare_dram_parameter("aw", [128, 4 * 8 * 128], BF16, isOutput=False)
    posb_e = nc.declare_dram_parameter("posb", [128, 4 * 256], F32, isOutput=False)
    ab_e = nc.declare_dram_parameter("ab", [128, 8], F32, isOutput=False)
    out_e = nc.declare_dram_parameter("out", [IMGS_PER_CORE, 64, 64], F32, isOutput=True)

    with TileContext(nc) as tc, ExitStack() as ctx:
        const = ctx.enter_context(tc.tile_pool(name="const", bufs=1))
        cw_sb = const.tile([128, 17 * 4 * 128], BF16, tag="cw")
        aw_sb = const.tile([128, 4 * 8 * 128], BF16, tag="aw")
        posb_sb = const.tile([128, 1024], F32, tag="posb")
        ab_sb = const.tile([128, 8], F32, tag="ab")
        idx_sb = const.tile([128, IMGS_PER_CORE * NCHUNK * (CHUNK // 16)], I16, tag="idx")
        nc.sync.dma_start(cw_sb[:], cw_e[:])
        nc.sync.dma_start(aw_sb[:], aw_e[:])
        nc.sync.dma_start(posb_sb[:], posb_e[:])
        nc.sync.dma_start(ab_sb[:], ab_e[:])

        idxv = idx_sb[:].rearrange("p (m c s) -> p m c s", m=IMGS_PER_CORE,
                                   c=NCHUNK, s=CHUNK // 16)
        for m in range(IMGS_PER_CORE):
            nc.sync.dma_start(idxv[:, m], idx_e[m])

        wp = ctx.enter_context(tc.tile_pool(name="wp", bufs=1))
        W_sb = wp.tile([128, IMGS_PER_CORE, NCHUNK, 2, CHUNK], BF16, tag="w")
        gp = ctx.enter_context(tc.tile_pool(name="gp", bufs=1))
        G = gp.tile([128, IMGS_PER_CORE, NCHUNK, 2, CHUNK], BF16, tag="g")
        fp = ctx.enter_context(tc.tile_pool(name="fp", bufs=1))
        F2 = fp.tile([128, IMGS_PER_CORE, NPTS], BF16, tag="f2")
        sp = ctx.enter_context(tc.tile_pool(name="sp", bufs=1))
        STG = sp.tile([128, IMGS_PER_CORE, NPTS], BF16, tag="stg")
        ODD = sp.tile([128, IMGS_PER_CORE, NPTS // 2], BF16, tag="odd")

        ktp = ctx.enter_context(tc.tile_pool(name="kt", bufs=1))
        # KT: [(s,ch) x (j(17), img(4), obj(64))]; j==16 is the norm tile
        KT = ktp.tile([128, 17, IMGS_PER_CORE, 64], BF16, tag="kt")
        nc.sync.dma_start(KT[:, 16, :, :], ktn_e[:])

        cfp = ctx.enter_context(tc.tile_pool(name="cfp", bufs=1))
        CF = cfp.tile([128, 4, 256], BF16, tag="cf")
        qkp = ctx.enter_context(tc.tile_pool(name="qkp", bufs=1))
        QK = qkp.tile([128, 8, 256], BF16, tag="qk")
        attp = ctx.enter_context(tc.tile_pool(name="attp", bufs=4))
        psp = ctx.enter_context(tc.tile_pool(name="psp", bufs=4, space="PSUM"))
        psap = ctx.enter_context(tc.tile_pool(name="psap", bufs=2, space="PSUM"))

        cwv = cw_sb[:].rearrange("p (j o m) -> p j o m", j=17, o=4, m=128)
        awv = aw_sb[:].rearrange("p (k m c) -> p k m c", k=4, m=8, c=128)
        posv = posb_sb[:].rearrange("p (o n) -> p o n", o=4, n=256)
        F2v = F2[:].rearrange("p m (s j t) -> p m s j t", s=2, j=16, t=64)
        STGv = STG[:].rearrange("p m (s j t) -> p m s j t", s=2, j=16, t=64)
        ODDv = ODD[:].rearrange("p m (j t) -> p m j t", j=16, t=64)

        F2c = F2[:].rearrange("p m (c i) -> p m c i", c=NCHUNK, i=CHUNK)
        for m in range(IMGS_PER_CORE):
            with nc.named_scope(f"wrep_{m}"):
                nc.sync.dma_start(W_sb[0:64, m], wrep_e[0, m].partition_broadcast(64))
                nc.sync.dma_start(W_sb[64:128, m], wrep_e[1, m].partition_broadcast(64))
            with nc.named_scope(f"gather_{m}"):
                for c in range(NCHUNK):
                    nc.gpsimd.dma_gather(
                        G[:, m, c], fmb_e[m], idxv[:, m, c],
                        CHUNK, CHUNK, 256, transpose=True)
            with nc.named_scope(f"combine_{m}"):
                # corner weights (broadcast over the 64-channel halves)
                nc.vector.tensor_tensor(G[:, m], G[:, m], W_sb[:, m], AL.mult)
                # y-corner add (free dim)
                for c in range(NCHUNK):
                    nc.vector.tensor_tensor(F2c[:, m, c], G[:, m, c, 0],
                                            G[:, m, c, 1], AL.add)
                # x-corner add: hi half (xoff=1) staged down to partitions 0:64
                nc.sync.dma_start(STG[0:64, m], F2[64:128, m])
                # even points -> KT[0:64], odd points -> staged then DMA up
                nc.vector.tensor_tensor(KT[0:64, 0:16, m, :], F2v[0:64, m, 0],
                                        STGv[0:64, m, 0], AL.add)
                nc.vector.tensor_tensor(ODDv[0:64, m], F2v[0:64, m, 1],
                                        STGv[0:64, m, 1], AL.add)
                nc.sync.dma_start(KT[64:128, 0:16, m, :], ODDv[0:64, m])

        # GEMM1 (conv): 4 M-chunks x 17 K-tiles, rhs 256 obj-cols
        with nc.named_scope("gemm1"):
            for o in range(4):
                ps = psp.tile([128, 256], F32, tag="ps")
                for j in range(17):
                    nc.tensor.matmul(ps[:], lhsT=cwv[:, j, o, :],
                                     rhs=KT[:, j, :, :],
                                     start=(j == 0), stop=(j == 16))
                nc.vector.tensor_tensor(CF[:, o], ps[:], posv[:, o], AL.add)

        # GEMM2 (attention in_proj)
        with nc.named_scope("gemm2"):
            for m8 in range(8):
                ps = psp.tile([128, 256], F32, tag="ps")
                for k in range(4):
                    nc.tensor.matmul(ps[:], lhsT=awv[:, k, m8, :],
                                     rhs=CF[:, k],
                                     start=(k == 0), stop=(k == 3))
                nc.scalar.activation(QK[:, m8], ps[:],
                                     AF.Identity, bias=ab_sb[:, m8:m8 + 1])

        # attention per image: p_w/sqrt(hd) folded into q rows on host, so
        # 4 accumulating K=128 matmuls sum over all heads.
        with nc.named_scope("attn"):
            for m in range(IMGS_PER_CORE):
                ps = psap.tile([64, 64], F32, tag="psa")
                for qc in range(4):
                    nc.tensor.matmul(ps[:],
                                     lhsT=QK[:, qc, m * 64:(m + 1) * 64],
                                     rhs=QK[:, 4 + qc, m * 64:(m + 1) * 64],
                                     start=(qc == 0), stop=(qc == 3))
                ATT = attp.tile([64, 64], F32, tag="att")
                nc.scalar.activation(ATT[:], ps[:], AF.Sigmoid)
                nc.sync.dma_start(out_e[m], ATT[:])

    nc.compile()
    _MODEL_CACHE["nc"] = nc
    return nc


def host_prep(inputs):
    """Host-side sharding + layout prep. Returns list of 8 per-core input maps."""
    import ml_dtypes
    bf = ml_dtypes.bfloat16

    cnn = np.ascontiguousarray(np.asarray(inputs["cnn_feature"], dtype=np.float32))
    contours = np.asarray(inputs["contours"], dtype=np.float32)
    ct_01 = np.asarray(inputs["ct_01"])
    ct_img_idx = np.asarray(inputs["ct_img_idx"])
    ct_ind = np.asarray(inputs["ct_ind"])
    h = int(inputs["h"]); w = int(inputs["w"])
    conv_w = np.asarray(inputs["conv_w"], dtype=np.float32)
    conv_b = np.asarray(inputs["conv_b"], dtype=np.float32)
    attn_w = np.asarray(inputs["attn_w"], dtype=np.float32)
    attn_b = np.asarray(inputs["attn_b"], dtype=np.float32)
    p_w = np.asarray(inputs["p_w"], dtype=np.float32)
    pos_embed = np.asarray(inputs["pos_embed"], dtype=np.float32)

    assert bool(np.all(ct_01)), "kernel requires ct_01 all ones"
    assert bool(np.all(ct_img_idx == np.repeat(np.arange(B, dtype=ct_img_idx.dtype), T)))

    # ---- 2x2-tile-block feature maps, 4 alignment copies ----------------
    # copy (sy,sx), block (ty,tx) holds pixels (2ty+sy+{0,1}, 2tx+sx+{0,1})
    # as [yoff, xoff, ch] bf16 (512B).  Zero padding beyond the image edge.
    c16 = cnn.astype(bf)                                # [32, 64, 160, 160]
    Pp = np.zeros((B, C, H + 2, W + 2), bf)
    Pp[:, :, :H, :W] = c16
    fmb = np.empty((B, 4, H // 2, W // 2, 2, 2, C), bf)
    for sy in range(2):
        for sx in range(2):
            sl = Pp[:, :, sy:sy + H, sx:sx + W].reshape(B, C, H // 2, 2, W // 2, 2)
            fmb[:, 2 * sy + sx] = sl.transpose(0, 2, 4, 3, 5, 1)
    fmb = fmb.reshape(B, NBLK, 256)

    # ---- per-point block index + slot weights ---------------------------
    cs = np.ascontiguousarray(contours[:, ::STRIDE])          # [N, 32, 2]
    px = cs[..., 0] * (float(W) / w) - 0.5
    py = cs[..., 1] * (float(H) / h) - 0.5
    x0 = np.floor(px); y0 = np.floor(py)
    wx = [x0 + 1.0 - px, px - x0]
    wy = [y0 + 1.0 - py, py - y0]
    cx = np.clip(x0, 0, W - 1).astype(np.int64)
    cy = np.clip(y0, 0, H - 1).astype(np.int64)
    sx = cx % 2; tx = (cx - sx) // 2
    sy = cy % 2; ty = (cy - sy) // 2
    blk = (sy * 2 + sx) * (H // 2 * (W // 2)) + ty * (W // 2) + tx  # [N, 32]
    x0i = x0.astype(np.int64); y0i = y0.astype(np.int64)

    w_slot = np.zeros((N_OBJ, P, 2, 2), np.float32)  # [n, p, yoff, xoff]
    for dy in range(2):
        for dx in range(2):
            ycorn = y0i + dy; xcorn = x0i + dx
            valid = (ycorn >= 0) & (ycorn < H) & (xcorn >= 0) & (xcorn < W)
            wgt = wy[dy] * wx[dx] * valid
            yoff = ycorn - cy; xoff = xcorn - cx
            for so in range(4):
                msk = valid & (yoff == so // 2) & (xoff == so % 2)
                w_slot[:, :, so // 2, so % 2] += np.where(msk, wgt, 0.0)

    normed = cs / np.array([w, h], np.float32)                # [N, 32, 2]

    ct_x = (ct_ind % W).astype(np.int64) * PATCH // W
    ct_y = (ct_ind // W).astype(np.int64) * PATCH // H
    posb_full = pos_embed[:, ct_y, ct_x] + conv_b[:, None]    # [512, N]

    s = np.ones(2 * NE, np.float32)
    s[:NE] = np.repeat(p_w[0, :, 0], NE // HEADS) / np.sqrt(np.float32(NE // HEADS))
    aw_t = (attn_w * s[:, None]).T                            # [512, 1024] (k, m)
    ab = attn_b * s                                           # [1024]

    # conv_w K-tiles -> cwT [128, 17*4*128]
    cw = np.zeros((17, 128, 512), np.float32)
    q = np.arange(128)
    for j in range(16):
        cw[j] = conv_w[:, q % 64, 2 * j + q // 64].T          # [128, 512]
    q64 = np.arange(64)
    cw[16, :64] = conv_w[:, 64 + q64 // 32, q64 % 32].T
    cwT = cw.reshape(17, 128, 4, 128).transpose(1, 0, 2, 3).reshape(128, 17 * 4 * 128)

    awT = aw_t.reshape(4, 128, 8, 128).transpose(1, 0, 2, 3).reshape(128, 4 * 8 * 128)
    abT = np.ascontiguousarray(ab.reshape(8, 128).T)          # [128, 8]

    in_maps = []
    for core in range(N_CORES):
        imgs = [IMGS_PER_CORE * core + i for i in range(IMGS_PER_CORE)]
        nbase = OBJS_PER_CORE * core

        # indices: per image, order i = (sp, j, t);  point p = 2j+sp
        bsel = blk[nbase:nbase + OBJS_PER_CORE].reshape(IMGS_PER_CORE, T, 16, 2)
        bord = bsel.transpose(0, 3, 2, 1).reshape(IMGS_PER_CORE, NPTS)  # [im,(sp,j,t)]
        idx = np.empty((IMGS_PER_CORE, 128, NCHUNK, CHUNK // 16), np.int16)
        for m in range(IMGS_PER_CORE):
            for c in range(NCHUNK):
                chunk = bord[m, c * CHUNK:(c + 1) * CHUNK]
                wrapped = chunk.reshape(CHUNK // 16, 16).T.astype(np.int16)
                idx[m, :, c, :] = np.tile(wrapped, (8, 1))
        idx = idx.reshape(IMGS_PER_CORE, 128, NCHUNK * (CHUNK // 16))

        # slot weights -> wrep [xoff, im, (chunk=(sp,j2), yoff, jj, t)]
        wsel = w_slot[nbase:nbase + OBJS_PER_CORE].reshape(
            IMGS_PER_CORE, T, 2, 8, 2, 2, 2)  # [im, t, j2, jj, sp, yoff, xoff]
        wrep = wsel.transpose(6, 0, 4, 2, 5, 3, 1).reshape(2, IMGS_PER_CORE, 2 * NPTS)

        # ktnorm [128, 256]: q<64: (coord=q//32, p=q%32); cols (im, t)
        ktn = np.zeros((128, 256), np.float32)
        ncols = nbase + np.arange(256)
        ktn[:64] = normed[ncols][:, np.arange(64) % 32, np.arange(64) // 32].T

        posbT = np.ascontiguousarray(
            posb_full[:, nbase:nbase + 256].reshape(4, 128, 256)
            .transpose(1, 0, 2).reshape(128, 1024))

        in_maps.append({
            "fmb": np.ascontiguousarray(fmb[imgs]),
            "idx": idx,
            "wrep": wrep.astype(bf),
            "ktn": ktn.astype(bf),
            "cw": cwT.astype(bf),
            "aw": awT.astype(bf),
            "posb": posbT.astype(np.float32),
            "ab": abT.astype(np.float32),
        })
    return in_maps


def run(in_maps, trace=False, **kw):
    nc = build_model()
    res = run_bass_kernel_spmd(nc, in_maps, core_ids=list(range(N_CORES)),
                               trace=trace, **kw)
    return res


def kernel(**inputs):
    in_maps = host_prep(inputs)
    res = run(in_maps)
    out = np.concatenate([res.results[i]["out"] for i in range(N_CORES)], axis=0)
    return out.astype(np.float32)


# revision 16
# speedup vs baseline: 2.9642x; 1.2531x over previous
"""Trainium2 Bass kernel for nn_AttentionCombine.

Self-contained: builds an SPMD Bass graph (same graph on 8 NeuronCores),
shards inputs data-parallel over the batch dim (4 images / 256 objects per
core), runs via run_bass_kernel_spmd, and reassembles the full output.

Gather strategy (v2): instead of loading full feature maps into SBUF and
using gpsimd ap_gather (~140us per gather), the host stores each image's
feature map in HBM as 2x2-pixel-tile blocks of 512B ([yoff(2), xoff(2),
ch(64)] bf16), replicated at the 4 (y,x) alignment parities. Any bilinear
2x2 footprint is then exactly ONE block, so one dma_gather(transpose=True)
index per contour point pulls all 4 corners x 64 channels straight from
HBM into SBUF in [partition=(xoff,ch), free=(yoff, point)] layout - the
exact layout the conv GEMM K-tiles need.

Per-core dataflow:
  - 4x dma_gather (one per image, 2048 idxs x 512B) on the SWDGE/pool path
  - corner-weight multiply on VectorE (weights host-computed, broadcast
    to the 64-partition halves), y-corner add in the free dim
  - x-corner add via one small SBUF->SBUF stage DMA + VectorE add,
    written into K-tile layout [(pt-parity, channel) x (j, img, obj)]
  - conv1d == GEMM over K=(66ch x 32pts) on TensorE (bf16, fp32 psum)
  - + positional embedding (host-gathered, device add)
  - qk GEMM (attention in_proj, p_w/sqrt(hd) folded into q rows on host)
  - attention per image: 4 accumulating K=128 matmuls
  - sigmoid on ScalarE, DMA out
"""
import os
import sys

for _p in ("/opt/trn_rl_repo", "/root/.axon_site/_ro/trn_rl_repo"):
    if os.path.isdir(_p) and _p not in sys.path:
        sys.path.append(_p)

import numpy as np
from contextlib import ExitStack

from concourse import bacc, mybir
from concourse.tile import TileContext
from concourse.bass_utils import run_bass_kernel_spmd

F32 = mybir.dt.float32
BF16 = mybir.dt.bfloat16
I16 = mybir.dt.int16

# Problem constants (hardcoded per spec)
B, C, H, W = 32, 64, 160, 160
IMG_HW = 640
N_OBJ = 2048
NUM_POINTS = 128
STRIDE = 4
P = NUM_POINTS // STRIDE  # 32 sampled points
NE = 512                  # n_embd
HEADS = 8
PATCH = 16
T = 64                    # objects per image
N_CORES = 8
IMGS_PER_CORE = B // N_CORES      # 4
OBJS_PER_CORE = N_OBJ // N_CORES  # 256
NPTS = P * T                      # 2048 gather points per image
NBLK = 4 * (H // 2) * (W // 2)    # 25600 tile-blocks per image
CHUNK = 512                       # idxs per dma_gather call
NCHUNK = NPTS // CHUNK            # 4 calls per image

_MODEL_CACHE = {}


def build_model():
    if "nc" in _MODEL_CACHE:
        return _MODEL_CACHE["nc"]
    nc = bacc.Bacc("TRN2", target_bir_lowering=False, debug=False)
    AL = mybir.AluOpType
    AF = mybir.ActivationFunctionType

    # fmb: per image, NBLK blocks of 256 bf16 = [yoff, xoff, ch]
    fmb_e = nc.declare_dram_parameter("fmb", [IMGS_PER_CORE, NBLK, 256], BF16, isOutput=False)
    # gather split into NCHUNK calls/image (SWDGE desc ring caps one call at
    # ~1008 idxs for 512B elems: rx descs/engine = n*2/16+2 must be <= 128)
    idx_e = nc.declare_dram_parameter(
        "idx", [IMGS_PER_CORE, 128, NCHUNK * (CHUNK // 16)], I16, isOutput=False)
    # wrep[xoff_half, img, (chunk, yoff, i512)]  (global i = sp*1024 + j*64 + t)
    wrep_e = nc.declare_dram_parameter("wrep", [2, IMGS_PER_CORE, 2 * NPTS], BF16, isOutput=False)
    ktn_e = nc.declare_dram_parameter("ktn", [128, 256], BF16, isOutput=False)
    cw_e = nc.declare_dram_parameter("cw", [128, 17 * 4 * 128], BF16, isOutput=False)
    aw_e = nc.declare_dram_parameter("aw", [128, 4 * 8 * 128], BF16, isOutput=False)
    posb_e = nc.declare_dram_parameter("posb", [128, 4 * 256], F32, isOutput=False)
    ab_e = nc.declare_dram_parameter("ab", [128, 8], F32, isOutput=False)
    out_e = nc.declare_dram_parameter("out", [IMGS_PER_CORE, 64, 64], F32, isOutput=True)

    with TileContext(nc) as tc, ExitStack() as ctx:
        const = ctx.enter_context(tc.tile_pool(name="const", bufs=1))
        cw_sb = const.tile([128, 17 * 4 * 128], BF16, tag="cw")
        aw_sb = const.tile([128, 4 * 8 * 128], BF16, tag="aw")
        posb_sb = const.tile([128, 1024], F32, tag="posb")
        ab_sb = const.tile([128, 8], F32, tag="ab")
        idx_sb = const.tile([128, IMGS_PER_CORE * NCHUNK * (CHUNK // 16)], I16, tag="idx")

        # idx first: the gathers gate on it; the big constants are needed
        # only at GEMM time.
        idxv = idx_sb[:].rearrange("p (m c s) -> p m c s", m=IMGS_PER_CORE,
                                   c=NCHUNK, s=CHUNK // 16)
        for m in range(IMGS_PER_CORE):
            nc.sync.dma_start(idxv[:, m], idx_e[m])
        nc.sync.dma_start(cw_sb[:], cw_e[:])
        nc.sync.dma_start(aw_sb[:], aw_e[:])
        nc.sync.dma_start(posb_sb[:], posb_e[:])
        nc.sync.dma_start(ab_sb[:], ab_e[:])

        wp = ctx.enter_context(tc.tile_pool(name="wp", bufs=1))
        W_sb = wp.tile([128, IMGS_PER_CORE, NCHUNK, 2, CHUNK], BF16, tag="w")
        gp = ctx.enter_context(tc.tile_pool(name="gp", bufs=1))
        G = gp.tile([128, IMGS_PER_CORE, NCHUNK, 2, CHUNK], BF16, tag="g")
        fp = ctx.enter_context(tc.tile_pool(name="fp", bufs=1))
        F2 = fp.tile([128, IMGS_PER_CORE, NPTS], BF16, tag="f2")
        sp = ctx.enter_context(tc.tile_pool(name="sp", bufs=1))
        STG = sp.tile([128, IMGS_PER_CORE, NPTS], BF16, tag="stg")
        ODD = sp.tile([128, IMGS_PER_CORE, NPTS // 2], BF16, tag="odd")

        ktp = ctx.enter_context(tc.tile_pool(name="kt", bufs=1))
        # KT: [(s,ch) x (j(17), img(4), obj(64))]; j==16 is the norm tile
        KT = ktp.tile([128, 17, IMGS_PER_CORE, 64], BF16, tag="kt")
        nc.sync.dma_start(KT[:, 16, :, :], ktn_e[:])

        cfp = ctx.enter_context(tc.tile_pool(name="cfp", bufs=1))
        CF = cfp.tile([128, 4, 256], BF16, tag="cf")
        qkp = ctx.enter_context(tc.tile_pool(name="qkp", bufs=1))
        QK = qkp.tile([128, 8, 256], BF16, tag="qk")
        attp = ctx.enter_context(tc.tile_pool(name="attp", bufs=4))
        psp = ctx.enter_context(tc.tile_pool(name="psp", bufs=4, space="PSUM"))
        psap = ctx.enter_context(tc.tile_pool(name="psap", bufs=2, space="PSUM"))

        cwv = cw_sb[:].rearrange("p (j o m) -> p j o m", j=17, o=4, m=128)
        awv = aw_sb[:].rearrange("p (k m c) -> p k m c", k=4, m=8, c=128)
        posv = posb_sb[:].rearrange("p (o n) -> p o n", o=4, n=256)


        F2c = F2[:].rearrange("p m (c i) -> p m c i", c=NCHUNK, i=CHUNK)
        STGc = STG[:].rearrange("p m (c i) -> p m c i", c=NCHUNK, i=CHUNK)
        JPC = CHUNK // T                 # j-tiles covered per chunk
        ODDj = ODD[:].rearrange("p m (j t) -> p m j t", j=16, t=64)
        for m in range(IMGS_PER_CORE):
            with nc.named_scope(f"wrep_{m}"):
                nc.sync.dma_start(W_sb[0:64, m], wrep_e[0, m].partition_broadcast(64))
                nc.sync.dma_start(W_sb[64:128, m], wrep_e[1, m].partition_broadcast(64))
            for c in range(NCHUNK):
                with nc.named_scope(f"gather_{m}_{c}"):
                    nc.gpsimd.dma_gather(
                        G[:, m, c], fmb_e[m], idxv[:, m, c],
                        CHUNK, CHUNK, 256, transpose=True)
                with nc.named_scope(f"combine_{m}_{c}"):
                    # corner weights (broadcast over the 64-channel halves),
                    # then y-corner add (free dim)
                    nc.vector.tensor_tensor(G[:, m, c], G[:, m, c],
                                            W_sb[:, m, c], AL.mult)
                    nc.vector.tensor_tensor(F2c[:, m, c], G[:, m, c, 0],
                                            G[:, m, c, 1], AL.add)
                    # x-corner add: hi half (xoff=1) staged to partitions 0:64
                    nc.sync.dma_start(STGc[0:64, m, c], F2c[64:128, m, c])
                    # chunk c covers point parity sp=c//2, j-tiles [jb, jb+JPC)
                    sp, jb = divmod(c, NCHUNK // 2)
                    jb *= JPC
                    if sp == 0:  # even points: straight into KT[0:64]
                        nc.vector.tensor_tensor(
                            KT[0:64, jb:jb + JPC, m, :],
                            F2c[0:64, m, c], STGc[0:64, m, c], AL.add)
                    else:        # odd points: stage then DMA to KT[64:128]
                        nc.vector.tensor_tensor(
                            ODDj[0:64, m, jb:jb + JPC, :],
                            F2c[0:64, m, c], STGc[0:64, m, c], AL.add)
                        nc.sync.dma_start(KT[64:128, jb:jb + JPC, m, :],
                                          ODDj[0:64, m, jb:jb + JPC, :])

        # GEMM1 (conv): 4 M-chunks x 17 K-tiles, rhs 256 obj-cols
        with nc.named_scope("gemm1"):
            for o in range(4):
                ps = psp.tile([128, 256], F32, tag="ps")
                for j in range(17):
                    nc.tensor.matmul(ps[:], lhsT=cwv[:, j, o, :],
                                     rhs=KT[:, j, :, :],
                                     start=(j == 0), stop=(j == 16))
                nc.vector.tensor_tensor(CF[:, o], ps[:], posv[:, o], AL.add)

        # GEMM2 (attention in_proj)
        with nc.named_scope("gemm2"):
            for m8 in range(8):
                ps = psp.tile([128, 256], F32, tag="ps")
                for k in range(4):
                    nc.tensor.matmul(ps[:], lhsT=awv[:, k, m8, :],
                                     rhs=CF[:, k],
                                     start=(k == 0), stop=(k == 3))
                nc.scalar.activation(QK[:, m8], ps[:],
                                     AF.Identity, bias=ab_sb[:, m8:m8 + 1])

        # attention per image: p_w/sqrt(hd) folded into q rows on host, so
        # 4 accumulating K=128 matmuls sum over all heads.
        with nc.named_scope("attn"):
            for m in range(IMGS_PER_CORE):
                ps = psap.tile([64, 64], F32, tag="psa")
                for qc in range(4):
                    nc.tensor.matmul(ps[:],
                                     lhsT=QK[:, qc, m * 64:(m + 1) * 64],
                                     rhs=QK[:, 4 + qc, m * 64:(m + 1) * 64],
                                     start=(qc == 0), stop=(qc == 3))
                ATT = attp.tile([64, 64], F32, tag="att")
                nc.scalar.activation(ATT[:], ps[:], AF.Sigmoid)
                nc.sync.dma_start(out_e[m], ATT[:])

    nc.compile()
    _MODEL_CACHE["nc"] = nc
    return nc


def host_prep(inputs):
    """Host-side sharding + layout prep. Returns list of 8 per-core input maps."""
    import ml_dtypes
    bf = ml_dtypes.bfloat16

    cnn = np.ascontiguousarray(np.asarray(inputs["cnn_feature"], dtype=np.float32))
    contours = np.asarray(inputs["contours"], dtype=np.float32)
    ct_01 = np.asarray(inputs["ct_01"])
    ct_img_idx = np.asarray(inputs["ct_img_idx"])
    ct_ind = np.asarray(inputs["ct_ind"])
    h = int(inputs["h"]); w = int(inputs["w"])
    conv_w = np.asarray(inputs["conv_w"], dtype=np.float32)
    conv_b = np.asarray(inputs["conv_b"], dtype=np.float32)
    attn_w = np.asarray(inputs["attn_w"], dtype=np.float32)
    attn_b = np.asarray(inputs["attn_b"], dtype=np.float32)
    p_w = np.asarray(inputs["p_w"], dtype=np.float32)
    pos_embed = np.asarray(inputs["pos_embed"], dtype=np.float32)

    assert bool(np.all(ct_01)), "kernel requires ct_01 all ones"
    assert bool(np.all(ct_img_idx == np.repeat(np.arange(B, dtype=ct_img_idx.dtype), T)))

    # ---- 2x2-tile-block feature maps, 4 alignment copies ----------------
    # copy (sy,sx), block (ty,tx) holds pixels (2ty+sy+{0,1}, 2tx+sx+{0,1})
    # as [yoff, xoff, ch] bf16 (512B).  Zero padding beyond the image edge.
    c16 = cnn.astype(bf)                                # [32, 64, 160, 160]
    Pp = np.zeros((B, C, H + 2, W + 2), bf)
    Pp[:, :, :H, :W] = c16
    fmb = np.empty((B, 4, H // 2, W // 2, 2, 2, C), bf)
    for sy in range(2):
        for sx in range(2):
            sl = Pp[:, :, sy:sy + H, sx:sx + W].reshape(B, C, H // 2, 2, W // 2, 2)
            fmb[:, 2 * sy + sx] = sl.transpose(0, 2, 4, 3, 5, 1)
    fmb = fmb.reshape(B, NBLK, 256)

    # ---- per-point block index + slot weights ---------------------------
    cs = np.ascontiguousarray(contours[:, ::STRIDE])          # [N, 32, 2]
    px = cs[..., 0] * (float(W) / w) - 0.5
    py = cs[..., 1] * (float(H) / h) - 0.5
    x0 = np.floor(px); y0 = np.floor(py)
    wx = [x0 + 1.0 - px, px - x0]
    wy = [y0 + 1.0 - py, py - y0]
    cx = np.clip(x0, 0, W - 1).astype(np.int64)
    cy = np.clip(y0, 0, H - 1).astype(np.int64)
    sx = cx % 2; tx = (cx - sx) // 2
    sy = cy % 2; ty = (cy - sy) // 2
    blk = (sy * 2 + sx) * (H // 2 * (W // 2)) + ty * (W // 2) + tx  # [N, 32]
    x0i = x0.astype(np.int64); y0i = y0.astype(np.int64)

    w_slot = np.zeros((N_OBJ, P, 2, 2), np.float32)  # [n, p, yoff, xoff]
    for dy in range(2):
        for dx in range(2):
            ycorn = y0i + dy; xcorn = x0i + dx
            valid = (ycorn >= 0) & (ycorn < H) & (xcorn >= 0) & (xcorn < W)
            wgt = wy[dy] * wx[dx] * valid
            yoff = ycorn - cy; xoff = xcorn - cx
            for so in range(4):
                msk = valid & (yoff == so // 2) & (xoff == so % 2)
                w_slot[:, :, so // 2, so % 2] += np.where(msk, wgt, 0.0)

    normed = cs / np.array([w, h], np.float32)                # [N, 32, 2]

    ct_x = (ct_ind % W).astype(np.int64) * PATCH // W
    ct_y = (ct_ind // W).astype(np.int64) * PATCH // H
    posb_full = pos_embed[:, ct_y, ct_x] + conv_b[:, None]    # [512, N]

    s = np.ones(2 * NE, np.float32)
    s[:NE] = np.repeat(p_w[0, :, 0], NE // HEADS) / np.sqrt(np.float32(NE // HEADS))
    aw_t = (attn_w * s[:, None]).T                            # [512, 1024] (k, m)
    ab = attn_b * s                                           # [1024]

    # conv_w K-tiles -> cwT [128, 17*4*128]
    cw = np.zeros((17, 128, 512), np.float32)
    q = np.arange(128)
    for j in range(16):
        cw[j] = conv_w[:, q % 64, 2 * j + q // 64].T          # [128, 512]
    q64 = np.arange(64)
    cw[16, :64] = conv_w[:, 64 + q64 // 32, q64 % 32].T
    cwT = cw.reshape(17, 128, 4, 128).transpose(1, 0, 2, 3).reshape(128, 17 * 4 * 128)

    awT = aw_t.reshape(4, 128, 8, 128).transpose(1, 0, 2, 3).reshape(128, 4 * 8 * 128)
    abT = np.ascontiguousarray(ab.reshape(8, 128).T)          # [128, 8]

    in_maps = []
    for core in range(N_CORES):
        imgs = [IMGS_PER_CORE * core + i for i in range(IMGS_PER_CORE)]
        nbase = OBJS_PER_CORE * core

        # indices: per image, order i = (sp, j, t);  point p = 2j+sp
        bsel = blk[nbase:nbase + OBJS_PER_CORE].reshape(IMGS_PER_CORE, T, 16, 2)
        bord = bsel.transpose(0, 3, 2, 1).reshape(IMGS_PER_CORE, NPTS)  # [im,(sp,j,t)]
        idx = np.empty((IMGS_PER_CORE, 128, NCHUNK, CHUNK // 16), np.int16)
        for m in range(IMGS_PER_CORE):
            for c in range(NCHUNK):
                chunk = bord[m, c * CHUNK:(c + 1) * CHUNK]
                wrapped = chunk.reshape(CHUNK // 16, 16).T.astype(np.int16)
                idx[m, :, c, :] = np.tile(wrapped, (8, 1))
        idx = idx.reshape(IMGS_PER_CORE, 128, NCHUNK * (CHUNK // 16))

        # slot weights -> wrep [xoff, im, (chunk=(sp,j2), yoff, jj, t)]
        wsel = w_slot[nbase:nbase + OBJS_PER_CORE].reshape(
            IMGS_PER_CORE, T, 2, 8, 2, 2, 2)  # [im, t, j2, jj, sp, yoff, xoff]
        wrep = wsel.transpose(6, 0, 4, 2, 5, 3, 1).reshape(2, IMGS_PER_CORE, 2 * NPTS)

        # ktnorm [128, 256]: q<64: (coord=q//32, p=q%32); cols (im, t)
        ktn = np.zeros((128, 256), np.float32)
        ncols = nbase + np.arange(256)
        ktn[:64] = normed[ncols][:, np.arange(64) % 32, np.arange(64) // 32].T

        posbT = np.ascontiguousarray(
            posb_full[:, nbase:nbase + 256].reshape(4, 128, 256)
            .transpose(1, 0, 2).reshape(128, 1024))

        in_maps.append({
            "fmb": np.ascontiguousarray(fmb[imgs]),
            "idx": idx,
            "wrep": wrep.astype(bf),
            "ktn": ktn.astype(bf),
            "cw": cwT.astype(bf),
            "aw": awT.astype(bf),
            "posb": posbT.astype(np.float32),
            "ab": abT.astype(np.float32),
        })
    return in_maps


def run(in_maps, trace=False, **kw):
    nc = build_model()
    res = run_bass_kernel_spmd(nc, in_maps, core_ids=list(range(N_CORES)),
                               trace=trace, **kw)
    return res


def kernel(**inputs):
    in_maps = host_prep(inputs)
    res = run(in_maps)
    out = np.concatenate([res.results[i]["out"] for i in range(N_CORES)], axis=0)
    return out.astype(np.float32)
